# Initial kernel scaffold
#
"""Trainium2 distributed kernel for the multi-query sparse-attention block.

Sharding: 8 cores = 2 batches x 4 head-groups (4 heads each).
J (key/value axis) is host-permuted to [self(2048) | ctx(256) | null(1) | pad(127)]
and the attention bias arrives pre-transposed (j-major), mask-folded and
pre-exponentiated in bf16:  attn_weight = exp(q.k) * expb.
Softmax runs without max-subtraction; the denominator comes from a ones-column
appended to V.  Output projection partials are ReduceScattered over each
4-core batch group, and the final layernorm runs on the scattered shards.
"""

import sys

sys.path.insert(0, "/opt/trn_rl_repo")

import numpy as np
import ml_dtypes

import concourse.bass as bass
import concourse.mybir as mybir
import concourse.tile as tile
from concourse import bacc
from concourse.bass_utils import run_bass_kernel_spmd
from concourse.masks import make_identity

F32 = mybir.dt.float32
F32R = mybir.dt.float32r
BF16 = mybir.dt.bfloat16
AF = mybir.ActivationFunctionType

B, N, D = 2, 2048, 1024
H, DH = 16, 64
C, CD = 256, 512
J = C + 1 + N          # 2305
JP = 19 * 128          # 2432 padded
HPC = 4                # heads per core
EPS = 1e-5

_cache = {}


def _ln_stats(nc, pool, src_ap, p, d):
    """mean/rstd/neg(mean*rstd) per partition for a [p, d] tile (d mult of 512)."""
    ns = d // 512
    stats = pool.tile([128, ns, 6], F32, tag="lnst")
    r = src_ap.rearrange("p (n f) -> p n f", f=512)
    for s in range(ns):
        nc.vector.bn_stats(out=stats[:p, s, :], in_=r[:, s, :])
    mv = pool.tile([128, 2], F32, tag="lnmv")
    nc.vector.bn_aggr(out=mv[:p, :], in_=stats[:p, :, :])
    mean = mv[:p, 0:1]
    rstd = pool.tile([128, 1], F32, tag="lnrs")
    nc.scalar.activation(rstd[:p, :], mv[:p, 1:2], AF.Sqrt, bias=EPS)
    nc.vector.reciprocal(rstd[:p, :], rstd[:p, :])
    negmr = pool.tile([128, 1], F32, tag="lnnm")
    nc.vector.tensor_scalar(
        out=negmr[:p, :], in0=mv[:p, 0:1], scalar1=rstd[:p, :], scalar2=-1.0,
        op0=mybir.AluOpType.mult, op1=mybir.AluOpType.mult)
    return mean, rstd, negmr


def build():
    nc = bacc.Bacc("TRN2", target_bir_lowering=False, debug=False, num_devices=8)

    expb = nc.declare_dram_parameter("expb", [HPC, JP, N], BF16, isOutput=False)
    x_in = nc.declare_dram_parameter("x", [N, D], F32, isOutput=False)
    ctx_in = nc.declare_dram_parameter("ctxt", [C, CD], F32, isOutput=False)
    nullk = nc.declare_dram_parameter("nullk", [DH, 1], F32, isOutput=False)
    nullv = nc.declare_dram_parameter("nullv", [1, DH], F32, isOutput=False)
    wq_in = nc.declare_dram_parameter("wq", [D, 256], F32, isOutput=False)
    wkv_in = nc.declare_dram_parameter("wkv", [D, 128], F32, isOutput=False)
    wctx_in = nc.declare_dram_parameter("wctx", [CD, 128], F32, isOutput=False)
    bctx_in = nc.declare_dram_parameter("bctx2", [1, 128], F32, isOutput=False)
    wout_in = nc.declare_dram_parameter("wout", [256, D], F32, isOutput=False)
    outg_in = nc.declare_dram_parameter("outg", [1, D], F32, isOutput=False)
    out_ext = nc.declare_dram_parameter("out", [N // 4, D], F32, isOutput=True)

    rs_in = nc.dram_tensor("rs_in", [N, D], F32)
    rs_out = nc.dram_tensor("rs_out", [N // 4, D], F32, addr_space="Shared")

    with tile.TileContext(nc) as tc:
        # ---------------- persistent tensors ----------------
        with tc.tile_pool(name="persist", bufs=1) as pp:
            wq_r = pp.tile([128, 8, 256], F32R)     # [Dc, qdim] chunks
            wkv_r = pp.tile([128, 8, 128], F32R)
            wctx_r = pp.tile([128, 4, 128], F32R)
            wout_r = pp.tile([128, 2, 1024], F32R)  # [hd-chunk, e]
            bctx_r = pp.tile([1, 128], F32R)
            ones_r = pp.tile([1, 1024], F32R)
            ident0 = pp.tile([128, 128], F32)
            ident_r = pp.tile([128, 128], F32R)
            gamma_bc = pp.tile([128, 1024], F32)
            qT = pp.tile([64, HPC * N], F32R)       # head h at cols h*N
            kT = pp.tile([64, JP], F32R)
            vext = pp.tile([128, 19 * 65], BF16)    # j-block jb at cols jb*65; col 64 = ones
            aoT0 = pp.tile([128, N], F32R)          # heads 0,1 of this core
            aoT1 = pp.tile([128, N], F32R)          # heads 2,3
            aoT = [aoT0, aoT1]

            # weight loads: gpsimd cast-DMA f32 -> f32r
            nc.gpsimd.dma_start(out=wq_r[:], in_=wq_in.rearrange("(c p) f -> p c f", p=128))
            nc.gpsimd.dma_start(out=wkv_r[:], in_=wkv_in.rearrange("(c p) f -> p c f", p=128))
            nc.gpsimd.dma_start(out=wctx_r[:], in_=wctx_in.rearrange("(c p) f -> p c f", p=128))
            nc.gpsimd.dma_start(out=wout_r[:], in_=wout_in.rearrange("(c p) f -> p c f", p=128))
            nc.gpsimd.dma_start(out=bctx_r[:], in_=bctx_in[:])
            nc.gpsimd.dma_start(out=kT[:, 2304:2305], in_=nullk[:])
            nc.gpsimd.dma_start(out=vext[0:1, 18 * 65:18 * 65 + 64], in_=nullv[:])

            zrow = pp.tile([128, 128], F32)
            nc.vector.memset(zrow[:], 0.0)
            nc.scalar.copy(kT[:, 2305:2432], zrow[0:64, 0:127])  # zero pad keys
            nc.vector.memset(vext[1:128, 18 * 65:18 * 65 + 64], 0.0)  # pad values
            for jb in range(19):
                nc.vector.memset(vext[:, jb * 65 + 64:jb * 65 + 65], 1.0)  # ones col

            o1 = pp.tile([1, 1024], F32)
            nc.vector.memset(o1[:], 1.0)
            nc.scalar.copy(ones_r[:], o1[:])
            make_identity(nc, ident0[:])
            nc.scalar.copy(ident_r[:], ident0[:])

            og_sb = pp.tile([1, 1024], F32)
            nc.sync.dma_start(out=og_sb[:], in_=outg_in[:])
            nc.gpsimd.partition_broadcast(gamma_bc[:], og_sb[:])

            # ---------------- context tokens -> kT/vext ----------------
            with tc.tile_pool(name="cwork", bufs=2) as cw, \
                 tc.tile_pool(name="cstat", bufs=2) as cs, \
                 tc.tile_pool(name="cps", bufs=2, space="PSUM") as cps:
                cnT = pp.tile([128, 4, 256], F32R)
                for t in range(2):
                    ct = cw.tile([128, CD], F32, tag="ct")
                    nc.sync.dma_start(out=ct[:], in_=ctx_in[t * 128:(t + 1) * 128, :])
                    mean, rstd, negmr = _ln_stats(nc, cs, ct[:], 128, CD)
                    cn = cw.tile([128, CD], F32R, tag="cn")
                    nc.scalar.activation(cn[:], ct[:], AF.Identity,
                                         bias=negmr[:128, :], scale=rstd[:128, :])
                    for c in range(4):
                        pt = cps.tile([128, 128], F32R, tag="ctp")
                        nc.tensor.matmul(pt[:], cn[:, c * 128:(c + 1) * 128],
                                         ident_r[:], is_transpose=True,
                                         start=True, stop=True)
                        nc.scalar.copy(cnT[:, c, t * 128:(t + 1) * 128],
                                       pt[:].bitcast(F32))
                # ck -> kT[:, 2048:2304]
                pck = cps.tile([64, 256], F32, tag="ck")
                for c in range(4):
                    nc.tensor.matmul(pck[:], wctx_r[:, c, 0:64], cnT[:, c, :],
                                     start=(c == 0), stop=False)
                nc.tensor.matmul(pck[:], bctx_r[:, 0:64], ones_r[:, 0:256],
                                 start=False, stop=True)
                nc.scalar.copy(kT[:, 2048:2304], pck[:])
                # cv -> vext blocks 16,17
                for t in range(2):
                    pcv = cps.tile([128, 64], F32, tag="cv")
                    for c in range(4):
                        nc.tensor.matmul(pcv[:], cnT[:, c, t * 128:(t + 1) * 128],
                                         wctx_r[:, c, 64:128],
                                         start=(c == 0), stop=False)
                    nc.tensor.matmul(pcv[:], ones_r[:, 0:128], bctx_r[:, 64:128],
                                     start=False, stop=True)
                    nc.vector.tensor_copy(vext[:, (16 + t) * 65:(16 + t) * 65 + 64],
                                          pcv[:])

            # ---------------- x: LN + transpose + projections ----------------
            with tc.tile_pool(name="xt", bufs=3) as xp, \
                 tc.tile_pool(name="xst", bufs=3) as xs, \
                 tc.tile_pool(name="xnt", bufs=2) as xnp, \
                 tc.tile_pool(name="vtmp", bufs=2) as vtp, \
                 tc.tile_pool(name="xps", bufs=2, space="PSUM") as xps, \
                 tc.tile_pool(name="pps", bufs=2, space="PSUM") as pps:
                for ic in range(4):
                    xnT = xnp.tile([128, 8, 512], F32R, tag="xnT")
                    for tb in range(4):
                        i0 = ic * 512 + tb * 128
                        xt = xp.tile([128, D], F32, tag="xt")
                        nc.sync.dma_start(out=xt[:], in_=x_in[i0:i0 + 128, :])
                        mean, rstd, negmr = _ln_stats(nc, xs, xt[:], 128, D)
                        xn = xp.tile([128, D], F32R, tag="xn")
                        nc.scalar.activation(xn[:], xt[:], AF.Identity,
                                             bias=negmr[:128, :], scale=rstd[:128, :])
                        for c in range(8):
                            pt = xps.tile([128, 128], F32R, tag="xtp")
                            nc.tensor.matmul(pt[:], xn[:, c * 128:(c + 1) * 128],
                                             ident_r[:], is_transpose=True,
                                             start=True, stop=True)
                            nc.scalar.copy(xnT[:, c, tb * 128:(tb + 1) * 128],
                                           pt[:].bitcast(F32))
                    # q projection: two 128-row blocks of q dims
                    for m in range(2):
                        pq = pps.tile([128, 512], F32, tag="pq")
                        for c in range(8):
                            nc.tensor.matmul(pq[:], wq_r[:, c, m * 128:(m + 1) * 128],
                                             xnT[:, c, :],
                                             start=(c == 0), stop=(c == 7))
                        for hh in range(2):
                            h = 2 * m + hh
                            nc.scalar.copy(
                                qT[:, h * N + ic * 512:h * N + ic * 512 + 512],
                                pq[hh * 64:hh * 64 + 64, :])
                    # k/v projection (joint): rows 0:64 k, 64:128 v
                    pkv = pps.tile([128, 512], F32, tag="pkv")
                    for c in range(8):
                        nc.tensor.matmul(pkv[:], wkv_r[:, c, :], xnT[:, c, :],
                                         start=(c == 0), stop=(c == 7))
                    nc.scalar.copy(kT[:, ic * 512:ic * 512 + 512], pkv[0:64, :])
                    vt = vtp.tile([64, 512], F32, tag="vt")
                    nc.scalar.copy(vt[:], pkv[64:128, :])
                    # transpose v to j-major -> vext
                    for tb in range(4):
                        pv = xps.tile([128, 64], F32, tag="vtp")
                        nc.tensor.matmul(pv[:], vt[:, tb * 128:(tb + 1) * 128],
                                         ident0[0:64, 0:64], is_transpose=True,
                                         start=True, stop=True)
                        jb = ic * 4 + tb
                        nc.vector.tensor_copy(vext[:, jb * 65:jb * 65 + 64], pv[:])

            # ---------------- attention ----------------
            with tc.tile_pool(name="eb", bufs=4) as ebp, \
                 tc.tile_pool(name="aw", bufs=3) as awp, \
                 tc.tile_pool(name="nrm", bufs=2) as nrm, \
                 tc.tile_pool(name="aps", bufs=2, space="PSUM") as aps, \
                 tc.tile_pool(name="ops", bufs=2, space="PSUM") as ops:
                for ih in range(2):
                    for h in range(HPC):
                        po = ops.tile([65, 1024], F32, tag="po")
                        for jb in range(19):
                            ps = aps.tile([128, 1024], F32, tag="ps")
                            for q in range(2):
                                nc.tensor.matmul(
                                    ps[:, q * 512:(q + 1) * 512],
                                    kT[:, jb * 128:(jb + 1) * 128],
                                    qT[:, h * N + ih * 1024 + q * 512:
                                       h * N + ih * 1024 + q * 512 + 512],
                                    start=True, stop=True)
                            eb = ebp.tile([128, 1024], BF16, tag="eb")
                            nc.sync.dma_start(
                                out=eb[:],
                                in_=expb[h, jb * 128:(jb + 1) * 128,
                                         ih * 1024:(ih + 1) * 1024])
                            et = awp.tile([128, 1024], BF16, tag="et")
                            nc.scalar.activation(et[:], ps[:], AF.Exp)
                            aw = awp.tile([128, 1024], BF16, tag="aw")
                            nc.vector.tensor_mul(aw[:], et[:], eb[:])
                            for q in range(2):
                                nc.tensor.matmul(
                                    po[:, q * 512:(q + 1) * 512],
                                    vext[:, jb * 65:jb * 65 + 65],
                                    aw[:, q * 512:(q + 1) * 512],
                                    start=(jb == 0), stop=(jb == 18))
                        # normalize: divide rows 0:64 by row 64
                        rec = nrm.tile([1, 1024], F32, tag="rec")
                        nc.vector.reciprocal(rec[:], po[64:65, :])
                        rec_r = nrm.tile([1, 1024], F32R, tag="recr")
                        nc.scalar.copy(rec_r[:], rec[:])
                        pbc = aps.tile([64, 1024], F32, tag="pbc")
                        nc.tensor.matmul(pbc[:], ones_r[:, 0:64], rec_r[:],
                                         start=True, stop=True)
                        rbc = nrm.tile([64, 1024], F32, tag="rbc")
                        nc.scalar.copy(rbc[:], pbc[:])
                        nc.vector.tensor_mul(
                            aoT[h // 2][(h % 2) * 64:(h % 2) * 64 + 64,
                                        ih * 1024:(ih + 1) * 1024],
                            po[0:64, :], rbc[:])

            # ---------------- output projection + reduce-scatter + LN ----------------
            with tc.tile_pool(name="ysb", bufs=3) as yp, \
                 tc.tile_pool(name="yps", bufs=3, space="PSUM") as yps:
                for ib in range(16):
                    y = yp.tile([128, 1024], F32, tag="y")
                    for ec in range(2):
                        py = yps.tile([128, 512], F32, tag="py")
                        for c in range(2):
                            nc.tensor.matmul(py[:], aoT[c][:, ib * 128:(ib + 1) * 128],
                                             wout_r[:, c, ec * 512:(ec + 1) * 512],
                                             start=(c == 0), stop=(c == 1))
                        if ec == 0:
                            nc.vector.tensor_copy(y[:, 0:512], py[:])
                        else:
                            nc.scalar.copy(y[:, 512:1024], py[:])
                    nc.sync.dma_start(out=rs_in[ib * 128:(ib + 1) * 128, :], in_=y[:])

                nc.gpsimd.collective_compute(
                    "ReduceScatter", mybir.AluOpType.add,
                    replica_groups=[[0, 1, 2, 3], [4, 5, 6, 7]],
                    ins=[rs_in[:]], outs=[rs_out[:]])

                with tc.tile_pool(name="fst", bufs=2) as fs:
                    for t in range(4):
                        ft = yp.tile([128, 1024], F32, tag="ft")
                        nc.sync.dma_start(out=ft[:],
                                          in_=rs_out[t * 128:(t + 1) * 128, :])
                        mean, rstd, negmr = _ln_stats(nc, fs, ft[:], 128, D)
                        fn = yp.tile([128, 1024], F32, tag="fn")
                        nc.scalar.activation(fn[:], ft[:], AF.Identity,
                                             bias=negmr[:128, :], scale=rstd[:128, :])
                        nc.vector.tensor_mul(fn[:], fn[:], gamma_bc[:])
                        nc.sync.dma_start(out=out_ext[t * 128:(t + 1) * 128, :],
                                          in_=fn[:])

    nc.compile()
    return nc


def _prep(inputs):
    x = np.asarray(inputs["x"], dtype=np.float32)
    context = np.asarray(inputs["context"], dtype=np.float32)
    mask = np.asarray(inputs["mask"])
    ab = np.asarray(inputs["attn_bias"], dtype=np.float32)
    norm_gamma = np.asarray(inputs["norm_gamma"], dtype=np.float32)
    null_kv = np.asarray(inputs["null_kv"], dtype=np.float32)
    Wq = np.asarray(inputs["Wq"], dtype=np.float32)
    Wkv = np.asarray(inputs["Wkv"], dtype=np.float32)
    ctx_ln_w = np.asarray(inputs["ctx_ln_w"], dtype=np.float32)
    ctx_ln_b = np.asarray(inputs["ctx_ln_b"], dtype=np.float32)
    Wctx = np.asarray(inputs["Wctx"], dtype=np.float32)
    bctx = np.asarray(inputs["bctx"], dtype=np.float32)
    Wout = np.asarray(inputs["Wout"], dtype=np.float32)
    out_gamma = np.asarray(inputs["out_gamma"], dtype=np.float32)

    scale = DH ** -0.5
    wq_f = (norm_gamma[:, None] * Wq) * scale            # (D, H*DH)
    wkv_f = np.ascontiguousarray(norm_gamma[:, None] * Wkv)
    wctx_f = np.ascontiguousarray(ctx_ln_w[:, None] * Wctx)
    bctx2 = np.ascontiguousarray((ctx_ln_b @ Wctx + bctx)[None, :])
    outg = np.ascontiguousarray(out_gamma[None, :])
    nullk = np.ascontiguousarray(null_kv[0][:, None])
    nullv = np.ascontiguousarray(null_kv[1][None, :])

    # J permute [self | ctx | null], transpose j-major, exponentiate
    bp = np.concatenate([ab[:, :, C + 1:], ab[:, :, :C + 1]], axis=2)
    ebT = np.exp(np.ascontiguousarray(bp.transpose(0, 2, 1)))  # (H, J, N) f32
    mvec = np.empty((B, J), dtype=np.float32)
    mvec[:, :N] = mask[:, C:]
    mvec[:, N:N + C] = mask[:, :C]
    mvec[:, N + C] = 1.0

    in_maps = []
    for core in range(8):
        b, g = core // 4, core % 4
        eb = ebT[HPC * g:HPC * g + HPC] * mvec[b][None, :, None]
        ebp = np.zeros((HPC, JP, N), dtype=ml_dtypes.bfloat16)
        ebp[:, :J, :] = eb.astype(ml_dtypes.bfloat16)
        in_maps.append({
            "expb": ebp,
            "x": np.ascontiguousarray(x[b]),
            "ctxt": np.ascontiguousarray(context[b]),
            "nullk": nullk,
            "nullv": nullv,
            "wq": np.ascontiguousarray(wq_f[:, 256 * g:256 * (g + 1)]),
            "wkv": wkv_f,
            "wctx": wctx_f,
            "bctx2": bctx2,
            "wout": np.ascontiguousarray(Wout[256 * g:256 * (g + 1), :]),
            "outg": outg,
        })
    return in_maps


def kernel(**inputs) -> np.ndarray:
    if "nc" not in _cache:
        _cache["nc"] = build()
    nc = _cache["nc"]
    in_maps = _prep(inputs)
    res = run_bass_kernel_spmd(nc, in_maps, core_ids=list(range(8))).results
    out = np.empty((B, N, D), dtype=np.float32)
    for core in range(8):
        b, r = core // 4, core % 4
        out[b, r * 512:(r + 1) * 512, :] = res[core]["out"]
    return out


# revision 7
# speedup vs baseline: 1.0388x; 1.0388x over previous
"""Trainium2 distributed kernel for the multi-query sparse-attention block.

Sharding: 8 cores = 2 batches x 4 head-groups (4 heads each).
J (key/value axis) is host-permuted to [self(2048) | ctx(256) | null(1) | pad(127)]
and the attention bias arrives pre-transposed (j-major), mask-folded and
pre-exponentiated in bf16:  attn_weight = exp(q.k) * expb.
Softmax runs without max-subtraction; the denominator comes from a ones-column
appended to V.  Output projection partials are ReduceScattered over each
4-core batch group, and the final layernorm runs on the scattered shards.
"""

import sys

sys.path.insert(0, "/opt/trn_rl_repo")

import numpy as np
import ml_dtypes

import concourse.bass as bass
import concourse.mybir as mybir
import concourse.tile as tile
from concourse import bacc
from concourse.bass_utils import run_bass_kernel_spmd
from concourse.masks import make_identity

F32 = mybir.dt.float32
F32R = mybir.dt.float32r
BF16 = mybir.dt.bfloat16
AF = mybir.ActivationFunctionType

B, N, D = 2, 2048, 1024
H, DH = 16, 64
C, CD = 256, 512
J = C + 1 + N          # 2305
JP = 19 * 128          # 2432 padded
HPC = 4                # heads per core
EPS = 1e-5

_cache = {}


def _ln_stats(nc, pool, src_ap, p, d, eps_ap):
    """mean/rstd/neg(mean*rstd) per partition for a [p, d] tile (d mult of 512)."""
    ns = d // 512
    stats = pool.tile([128, ns, 6], F32, tag="lnst")
    r = src_ap.rearrange("p (n f) -> p n f", f=512)
    for s in range(ns):
        nc.vector.bn_stats(out=stats[:p, s, :], in_=r[:, s, :])
    mv = pool.tile([128, 2], F32, tag="lnmv")
    nc.vector.bn_aggr(out=mv[:p, :], in_=stats[:p, :, :])
    mean = mv[:p, 0:1]
    rstd = pool.tile([128, 1], F32, tag="lnrs")
    nc.scalar.activation(rstd[:p, :], mv[:p, 1:2], AF.Sqrt, bias=eps_ap[:p, :])
    nc.vector.reciprocal(rstd[:p, :], rstd[:p, :])
    negmr = pool.tile([128, 1], F32, tag="lnnm")
    nc.vector.tensor_scalar(
        out=negmr[:p, :], in0=mv[:p, 0:1], scalar1=rstd[:p, :], scalar2=-1.0,
        op0=mybir.AluOpType.mult, op1=mybir.AluOpType.mult)
    return mean, rstd, negmr


def build():
    nc = bacc.Bacc("TRN2", target_bir_lowering=False, debug=False, num_devices=8)

    expb = nc.declare_dram_parameter("expb", [HPC, JP, N], BF16, isOutput=False)
    x_in = nc.declare_dram_parameter("x", [N, D], F32, isOutput=False)
    ctx_in = nc.declare_dram_parameter("ctxt", [C, CD], F32, isOutput=False)
    nullk = nc.declare_dram_parameter("nullk", [DH, 1], F32, isOutput=False)
    nullv = nc.declare_dram_parameter("nullv", [1, DH], F32, isOutput=False)
    wq_in = nc.declare_dram_parameter("wq", [D, 256], F32, isOutput=False)
    wkv_in = nc.declare_dram_parameter("wkv", [D, 128], F32, isOutput=False)
    wctx_in = nc.declare_dram_parameter("wctx", [CD, 128], F32, isOutput=False)
    bctx_in = nc.declare_dram_parameter("bctx2", [1, 128], F32, isOutput=False)
    wout_in = nc.declare_dram_parameter("wout", [256, D], F32, isOutput=False)
    outg_in = nc.declare_dram_parameter("outg", [1, D], F32, isOutput=False)
    out_ext = nc.declare_dram_parameter("out", [N // 4, D], F32, isOutput=True)

    rs_in = nc.dram_tensor("rs_in", [N, D], F32)
    rs_out = nc.dram_tensor("rs_out", [N // 4, D], F32)

    with tile.TileContext(nc) as tc:
        # ---------------- persistent tensors ----------------
        with tc.tile_pool(name="persist", bufs=1) as pp:
            wq_r = pp.tile([128, 8, 256], F32R)     # [Dc, qdim] chunks
            wkv_r = pp.tile([128, 8, 128], F32R)
            wctx_r = pp.tile([128, 4, 128], F32R)
            wout_r = pp.tile([128, 2, 1024], F32R)  # [hd-chunk, e]
            bctx_r = pp.tile([1, 128], F32R)
            ones_r = pp.tile([1, 1024], F32R)
            ident0 = pp.tile([128, 128], F32)
            ident_r = pp.tile([128, 128], F32R)
            gamma_bc = pp.tile([128, 1024], F32)
            qT = pp.tile([64, HPC * N], F32R)       # head h at cols h*N
            kT = pp.tile([64, JP], F32R)
            vext = pp.tile([128, 19 * 65], BF16)    # j-block jb at cols jb*65; col 64 = ones
            aoT0 = pp.tile([128, N], F32R)          # heads 0,1 of this core
            aoT1 = pp.tile([128, N], F32R)          # heads 2,3
            aoT = [aoT0, aoT1]

            # weight loads: gpsimd cast-DMA f32 -> f32r
            nc.gpsimd.dma_start(out=wq_r[:], in_=wq_in.rearrange("(c p) f -> p c f", p=128))
            nc.gpsimd.dma_start(out=wkv_r[:], in_=wkv_in.rearrange("(c p) f -> p c f", p=128))
            nc.gpsimd.dma_start(out=wctx_r[:], in_=wctx_in.rearrange("(c p) f -> p c f", p=128))
            nc.gpsimd.dma_start(out=wout_r[:], in_=wout_in.rearrange("(c p) f -> p c f", p=128))
            nc.gpsimd.dma_start(out=bctx_r[:], in_=bctx_in[:])
            nc.gpsimd.dma_start(out=kT[:, 2304:2305], in_=nullk[:])

            eps_t = pp.tile([128, 1], F32)
            nc.gpsimd.memset(eps_t[:], EPS)
            zrow = pp.tile([128, 128], F32)
            nc.vector.memset(zrow[:], 0.0)
            nc.scalar.copy(kT[:, 2305:2432], zrow[0:64, 0:127])  # zero pad keys
            nc.vector.memset(vext[:, 18 * 65:18 * 65 + 64], 0.0)  # pad values
            nc.gpsimd.dma_start(out=vext[0:1, 18 * 65:18 * 65 + 64], in_=nullv[:])
            for jb in range(19):
                nc.vector.memset(vext[:, jb * 65 + 64:jb * 65 + 65], 1.0)  # ones col

            o1 = pp.tile([1, 1024], F32)
            nc.vector.memset(o1[:], 1.0)
            nc.scalar.copy(ones_r[:], o1[:])
            make_identity(nc, ident0[:])
            nc.scalar.copy(ident_r[:], ident0[:])

            og_sb = pp.tile([1, 1024], F32)
            nc.sync.dma_start(out=og_sb[:], in_=outg_in[:])
            nc.gpsimd.partition_broadcast(gamma_bc[:], og_sb[:])

            # ---------------- context tokens -> kT/vext ----------------
            with tc.tile_pool(name="cwork", bufs=2) as cw, \
                 tc.tile_pool(name="cstat", bufs=2) as cs, \
                 tc.tile_pool(name="cps", bufs=2, space="PSUM") as cps:
                cnT = pp.tile([128, 4, 256], F32R)
                for t in range(2):
                    ct = cw.tile([128, CD], F32, tag="ct")
                    nc.sync.dma_start(out=ct[:], in_=ctx_in[t * 128:(t + 1) * 128, :])
                    mean, rstd, negmr = _ln_stats(nc, cs, ct[:], 128, CD, eps_t)
                    cn = cw.tile([128, CD], F32R, tag="cn")
                    nc.scalar.activation(cn[:], ct[:], AF.Identity,
                                         bias=negmr[:128, :], scale=rstd[:128, :])
                    for c in range(4):
                        pt = cps.tile([128, 128], F32R, tag="ctp")
                        nc.tensor.matmul(pt[:], cn[:, c * 128:(c + 1) * 128],
                                         ident_r[:], is_transpose=True,
                                         start=True, stop=True)
                        nc.scalar.copy(cnT[:, c, t * 128:(t + 1) * 128],
                                       pt[:].bitcast(F32))
                # ck -> kT[:, 2048:2304]
                pck = cps.tile([64, 256], F32, tag="ck")
                for c in range(4):
                    nc.tensor.matmul(pck[:], wctx_r[:, c, 0:64], cnT[:, c, :],
                                     start=(c == 0), stop=False)
                nc.tensor.matmul(pck[:], bctx_r[:, 0:64], ones_r[:, 0:256],
                                 start=False, stop=True)
                nc.scalar.copy(kT[:, 2048:2304], pck[:])
                # cv -> vext blocks 16,17
                for t in range(2):
                    pcv = cps.tile([128, 64], F32, tag="cv")
                    for c in range(4):
                        nc.tensor.matmul(pcv[:], cnT[:, c, t * 128:(t + 1) * 128],
                                         wctx_r[:, c, 64:128],
                                         start=(c == 0), stop=False)
                    nc.tensor.matmul(pcv[:], ones_r[:, 0:128], bctx_r[:, 64:128],
                                     start=False, stop=True)
                    nc.vector.tensor_copy(vext[:, (16 + t) * 65:(16 + t) * 65 + 64],
                                          pcv[:])

            # ---------------- x: LN + transpose + projections ----------------
            with tc.tile_pool(name="xt", bufs=3) as xp, \
                 tc.tile_pool(name="xst", bufs=3) as xs, \
                 tc.tile_pool(name="xnt", bufs=2) as xnp, \
                 tc.tile_pool(name="vtmp", bufs=2) as vtp, \
                 tc.tile_pool(name="xps", bufs=2, space="PSUM") as xps, \
                 tc.tile_pool(name="pps", bufs=2, space="PSUM") as pps:
                for ic in range(4):
                    xnT = xnp.tile([128, 8, 512], F32R, tag="xnT")
                    for tb in range(4):
                        i0 = ic * 512 + tb * 128
                        xt = xp.tile([128, D], F32, tag="xt")
                        nc.sync.dma_start(out=xt[:], in_=x_in[i0:i0 + 128, :])
                        mean, rstd, negmr = _ln_stats(nc, xs, xt[:], 128, D, eps_t)
                        xn = xp.tile([128, D], F32R, tag="xn")
                        nc.scalar.activation(xn[:], xt[:], AF.Identity,
                                             bias=negmr[:128, :], scale=rstd[:128, :])
                        for c in range(8):
                            pt = xps.tile([128, 128], F32R, tag="xtp")
                            nc.tensor.matmul(pt[:], xn[:, c * 128:(c + 1) * 128],
                                             ident_r[:], is_transpose=True,
                                             start=True, stop=True)
                            nc.scalar.copy(xnT[:, c, tb * 128:(tb + 1) * 128],
                                           pt[:].bitcast(F32))
                    # q projection: two 128-row blocks of q dims
                    for m in range(2):
                        pq = pps.tile([128, 512], F32, tag="pq")
                        for c in range(8):
                            nc.tensor.matmul(pq[:], wq_r[:, c, m * 128:(m + 1) * 128],
                                             xnT[:, c, :],
                                             start=(c == 0), stop=(c == 7))
                        for hh in range(2):
                            h = 2 * m + hh
                            nc.scalar.copy(
                                qT[:, h * N + ic * 512:h * N + ic * 512 + 512],
                                pq[hh * 64:hh * 64 + 64, :])
                    # k/v projection (joint): rows 0:64 k, 64:128 v
                    pkv = pps.tile([128, 512], F32, tag="pkv")
                    for c in range(8):
                        nc.tensor.matmul(pkv[:], wkv_r[:, c, :], xnT[:, c, :],
                                         start=(c == 0), stop=(c == 7))
                    nc.scalar.copy(kT[:, ic * 512:ic * 512 + 512], pkv[0:64, :])
                    vt = vtp.tile([64, 512], F32, tag="vt")
                    nc.scalar.copy(vt[:], pkv[64:128, :])
                    # transpose v to j-major -> vext
                    for tb in range(4):
                        pv = xps.tile([128, 64], F32, tag="vtp")
                        nc.tensor.matmul(pv[:], vt[:, tb * 128:(tb + 1) * 128],
                                         ident0[0:64, 0:64], is_transpose=True,
                                         start=True, stop=True)
                        jb = ic * 4 + tb
                        nc.vector.tensor_copy(vext[:, jb * 65:jb * 65 + 64], pv[:])

            # ---------------- attention ----------------
            with tc.tile_pool(name="eb", bufs=4) as ebp, \
                 tc.tile_pool(name="aw", bufs=3) as awp, \
                 tc.tile_pool(name="nrm", bufs=2) as nrm, \
                 tc.tile_pool(name="aps", bufs=2, space="PSUM") as aps, \
                 tc.tile_pool(name="ops", bufs=2, space="PSUM") as ops:
                for ih in range(2):
                    for h in range(HPC):
                        po = ops.tile([65, 1024], F32, tag="po")
                        for jb in range(19):
                            ps = aps.tile([128, 1024], F32, tag="ps")
                            for q in range(2):
                                nc.tensor.matmul(
                                    ps[:, q * 512:(q + 1) * 512],
                                    kT[:, jb * 128:(jb + 1) * 128],
                                    qT[:, h * N + ih * 1024 + q * 512:
                                       h * N + ih * 1024 + q * 512 + 512],
                                    start=True, stop=True)
                            eb = ebp.tile([128, 1024], BF16, tag="eb")
                            nc.sync.dma_start(
                                out=eb[:],
                                in_=expb[h, jb * 128:(jb + 1) * 128,
                                         ih * 1024:(ih + 1) * 1024])
                            et = awp.tile([128, 1024], BF16, tag="et")
                            nc.scalar.activation(et[:], ps[:], AF.Exp)
                            aw = awp.tile([128, 1024], BF16, tag="aw")
                            nc.vector.tensor_mul(aw[:], et[:], eb[:])
                            for q in range(2):
                                nc.tensor.matmul(
                                    po[:, q * 512:(q + 1) * 512],
                                    vext[:, jb * 65:jb * 65 + 65],
                                    aw[:, q * 512:(q + 1) * 512],
                                    start=(jb == 0), stop=(jb == 18))
                        # normalize: divide rows 0:64 by row 64
                        rec = nrm.tile([1, 1024], F32, tag="rec")
                        nc.vector.reciprocal(rec[:], po[64:65, :])
                        rec_r = nrm.tile([1, 1024], F32R, tag="recr")
                        nc.scalar.copy(rec_r[:], rec[:])
                        pbc = aps.tile([128, 1024], F32, tag="ps")
                        for q in range(2):
                            nc.tensor.matmul(pbc[0:64, q * 512:(q + 1) * 512],
                                             ones_r[:, 0:64],
                                             rec_r[:, q * 512:(q + 1) * 512],
                                             start=True, stop=True)
                        rbc = nrm.tile([64, 1024], F32, tag="rbc")
                        nc.scalar.copy(rbc[:], pbc[0:64, :])
                        nc.vector.tensor_mul(
                            aoT[h // 2][(h % 2) * 64:(h % 2) * 64 + 64,
                                        ih * 1024:(ih + 1) * 1024],
                            po[0:64, :], rbc[:])

            # ---------------- output projection + reduce-scatter + LN ----------------
            with tc.tile_pool(name="ysb", bufs=3) as yp, \
                 tc.tile_pool(name="yps", bufs=3, space="PSUM") as yps:
                for ib in range(16):
                    y = yp.tile([128, 1024], F32, tag="y")
                    for ec in range(2):
                        py = yps.tile([128, 512], F32, tag="py")
                        for c in range(2):
                            nc.tensor.matmul(py[:], aoT[c][:, ib * 128:(ib + 1) * 128],
                                             wout_r[:, c, ec * 512:(ec + 1) * 512],
                                             start=(c == 0), stop=(c == 1))
                        if ec == 0:
                            nc.vector.tensor_copy(y[:, 0:512], py[:])
                        else:
                            nc.scalar.copy(y[:, 512:1024], py[:])
                    nc.sync.dma_start(out=rs_in[ib * 128:(ib + 1) * 128, :], in_=y[:])

                nc.gpsimd.collective_compute(
                    "ReduceScatter", mybir.AluOpType.add,
                    replica_groups=[[0, 1, 2, 3], [4, 5, 6, 7]],
                    ins=[rs_in[:]], outs=[rs_out[:]])

                with tc.tile_pool(name="fst", bufs=2) as fs:
                    for t in range(4):
                        ft = yp.tile([128, 1024], F32, tag="ft")
                        nc.sync.dma_start(out=ft[:],
                                          in_=rs_out[t * 128:(t + 1) * 128, :])
                        mean, rstd, negmr = _ln_stats(nc, fs, ft[:], 128, D, eps_t)
                        fn = yp.tile([128, 1024], F32, tag="fn")
                        nc.scalar.activation(fn[:], ft[:], AF.Identity,
                                             bias=negmr[:128, :], scale=rstd[:128, :])
                        nc.vector.tensor_mul(fn[:], fn[:], gamma_bc[:])
                        nc.sync.dma_start(out=out_ext[t * 128:(t + 1) * 128, :],
                                          in_=fn[:])

    nc.compile()
    return nc


def _prep(inputs):
    x = np.asarray(inputs["x"], dtype=np.float32)
    context = np.asarray(inputs["context"], dtype=np.float32)
    mask = np.asarray(inputs["mask"])
    ab = np.asarray(inputs["attn_bias"], dtype=np.float32)
    norm_gamma = np.asarray(inputs["norm_gamma"], dtype=np.float32)
    null_kv = np.asarray(inputs["null_kv"], dtype=np.float32)
    Wq = np.asarray(inputs["Wq"], dtype=np.float32)
    Wkv = np.asarray(inputs["Wkv"], dtype=np.float32)
    ctx_ln_w = np.asarray(inputs["ctx_ln_w"], dtype=np.float32)
    ctx_ln_b = np.asarray(inputs["ctx_ln_b"], dtype=np.float32)
    Wctx = np.asarray(inputs["Wctx"], dtype=np.float32)
    bctx = np.asarray(inputs["bctx"], dtype=np.float32)
    Wout = np.asarray(inputs["Wout"], dtype=np.float32)
    out_gamma = np.asarray(inputs["out_gamma"], dtype=np.float32)

    scale = DH ** -0.5
    wq_f = (norm_gamma[:, None] * Wq) * scale            # (D, H*DH)
    wkv_f = np.ascontiguousarray(norm_gamma[:, None] * Wkv)
    wctx_f = np.ascontiguousarray(ctx_ln_w[:, None] * Wctx)
    bctx2 = np.ascontiguousarray((ctx_ln_b @ Wctx + bctx)[None, :])
    outg = np.ascontiguousarray(out_gamma[None, :])
    nullk = np.ascontiguousarray(null_kv[0][:, None])
    nullv = np.ascontiguousarray(null_kv[1][None, :])

    # J permute [self | ctx | null], transpose j-major, exponentiate
    bp = np.concatenate([ab[:, :, C + 1:], ab[:, :, :C + 1]], axis=2)
    ebT = np.exp(np.ascontiguousarray(bp.transpose(0, 2, 1)))  # (H, J, N) f32
    mvec = np.empty((B, J), dtype=np.float32)
    mvec[:, :N] = mask[:, C:]
    mvec[:, N] = 1.0                       # ctx[0]: the left-pad True
    mvec[:, N + 1:N + C] = mask[:, :C - 1]  # ctx[c] <- mask[c-1]
    mvec[:, N + C] = mask[:, C - 1]         # null <- mask[255]

    in_maps = []
    for core in range(8):
        b, g = core // 4, core % 4
        eb = ebT[HPC * g:HPC * g + HPC] * mvec[b][None, :, None]
        ebp = np.zeros((HPC, JP, N), dtype=ml_dtypes.bfloat16)
        ebp[:, :J, :] = eb.astype(ml_dtypes.bfloat16)
        in_maps.append({
            "expb": ebp,
            "x": np.ascontiguousarray(x[b]),
            "ctxt": np.ascontiguousarray(context[b]),
            "nullk": nullk,
            "nullv": nullv,
            "wq": np.ascontiguousarray(wq_f[:, 256 * g:256 * (g + 1)]),
            "wkv": wkv_f,
            "wctx": wctx_f,
            "bctx2": bctx2,
            "wout": np.ascontiguousarray(Wout[256 * g:256 * (g + 1), :]),
            "outg": outg,
        })
    return in_maps


def kernel(**inputs) -> np.ndarray:
    if "nc" not in _cache:
        _cache["nc"] = build()
    nc = _cache["nc"]
    in_maps = _prep(inputs)
    res = run_bass_kernel_spmd(nc, in_maps, core_ids=list(range(8))).results
    out = np.empty((B, N, D), dtype=np.float32)
    for core in range(8):
        b, r = core // 4, core % 4
        out[b, r * 512:(r + 1) * 512, :] = res[core]["out"]
    return out
